# revision 32
# baseline (speedup 1.0000x reference)
"""GCNConv (SpMM + dense projection) Trainium2 Bass kernel, 8-core SPMD.

Math: out = A @ x @ W, A symmetric COO (row, col, values), N=100000 nodes,
F=128 features, 1.6M edges.

Distribution (CAGNET-style 1D row partition): core m owns destination rows
[m*12500, (m+1)*12500). x is replicated in every core's HBM; each core
gathers the source rows its edges need via dma_gather (fp16, 256B rows).

Per-core device pipeline (single pass, pipelined by Tile):
  1. dma_gather x[col] rows for a group of 256-dest windows, one call per
     (window-group, col-chunk) — gather indices are int16 so x is split in
     4 chunks of 25000 rows. SWDGE descriptor generation is the serial
     critical path (~2.5 ns/row on the queue's Q7 pair; GpSimd custom ops
     never overlap — the Pool NX waits for all 8 cores' rsp_done).
  2. The per-tile scatter matrices S[k, j] = v_k * (winslot(dest_k) == j)
     are PREBUILT ON HOST (dense [128, 256] fp16 blocks of the window-
     bucketed adjacency) and streamed from HBM via HWDGE (nc.sync) in
     half-call pieces — this keeps the DVE idle, which also speeds up
     SWDGE desc-gen (shared SBUF ports).
  3. PE matmul gt_tile^T @ S_tile accumulates z^T[feat, dest] into a PSUM
     region per 256-dest window.
  4. Evict PSUM -> SBUF fp16 (scalar engine), multiply by W (PE, W
     stationary) -> out^T, evict, DMA out.

Host work: capacity-skewed window load balancing (early windows heavier so
the pipeline drain tail is short; all cores share one static tile
schedule), bucketing edges by (core, window, chunk), packing the gather
index stream and scatter-matrix blocks, and the final unpermute + cast.
"""
import sys

if "/opt/trn_rl_repo" not in sys.path:
    sys.path.insert(0, "/opt/trn_rl_repo")

import numpy as np
from contextlib import ExitStack

import concourse.bacc as bacc
import concourse.tile as tile
import concourse.mybir as mybir
from concourse import bass_utils

F16 = mybir.dt.float16
F32 = mybir.dt.float32
I16 = mybir.dt.int16

# ---------------------------------------------------------------- config ---
DEFAULT_CFG = dict(
    n_nodes=100000,
    feat=128,
    n_cores=8,
    npc=12500,       # destination rows per core
    n_chunk=4,       # x row chunks (gather idx must fit int16)
    ch_rows=25000,   # rows per chunk
    wdest=256,       # dests per window (= half a PSUM bank of fp32)
    nw=52,           # windows per core (52*256 = 13312 >= 12500)
    wpair=4,         # windows whose gathers are batched into one call
)


# ------------------------------------------------------- host preprocessing
def _assign_windows(deg4, nw, wdest):
    """Balanced assignment of destinations to nw windows (<= wdest each).

    Batched LPT: heaviest remaining dests go to the windows with the
    smallest worst-chunk load. Returns (win, slot) per destination.
    """
    npc = deg4.shape[0]
    tot = deg4.sum(axis=1)
    order = np.argsort(-tot, kind="stable")
    win = np.empty(npc, np.int32)
    slot = np.empty(npc, np.int32)
    loads = np.zeros((nw, 4), np.int64)
    nslot = np.zeros(nw, np.int32)
    # capacity ramp: early windows take more edges so the final batches
    # (processed last) are light and the pipeline drain tail is short
    cap = np.linspace(1.35, 0.45, nw)
    pos = 0
    while pos < npc:
        k = min(nw, npc - pos)
        batch = order[pos : pos + k]
        wsel = np.argsort(loads.max(axis=1) / cap, kind="stable")[:k].astype(
            np.int32
        )
        win[batch] = wsel
        slot[batch] = nslot[wsel]
        nslot[wsel] += 1
        loads[wsel] += deg4[batch]
        pos += k
    assert nslot.max() <= wdest, f"window overflow: {nslot.max()}"
    return win, slot


def _preprocess(row, col, values, cfg):
    """Bucket edges per (core, window, chunk); compute the shared static tile
    schedule T[w][c]; pack per-core gather/slot/value streams."""
    nc_ = cfg["n_cores"]
    npc = cfg["npc"]
    chr_ = cfg["ch_rows"]
    nw = cfg["nw"]
    wdest = cfg["wdest"]
    wpair = cfg["wpair"]

    core = row // npc
    per_core = []
    for m in range(nc_):
        sel = np.flatnonzero(core == m)
        dl = (row[sel] - m * npc).astype(np.int64)
        cc = (col[sel] // chr_).astype(np.int64)
        lc = (col[sel] - cc * chr_).astype(np.int64)
        vv = values[sel].astype(np.float32)
        deg4 = np.bincount(dl * 4 + cc, minlength=npc * 4).reshape(npc, 4)
        win, slot = _assign_windows(deg4, nw, wdest)
        counts = np.bincount(
            win[dl].astype(np.int64) * 4 + cc, minlength=nw * 4
        ).reshape(nw, 4)
        per_core.append(dict(dl=dl, cc=cc, lc=lc, vv=vv, win=win, slot=slot,
                             counts=counts))

    # shared static schedule
    cmax = np.stack([pc["counts"] for pc in per_core]).max(axis=0)
    T = np.maximum((cmax + 127) // 128, 1).astype(np.int64)  # [nw, 4] tiles

    # stream layout: for window-pair b, for chunk c, for w in pair: T[w][c]
    n_batch = nw // wpair
    offs = np.zeros((nw, 4), np.int64)
    call_tiles = np.zeros((n_batch, 4), np.int64)
    cum = 0
    for b in range(n_batch):
        for c in range(4):
            for w in range(b * wpair, (b + 1) * wpair):
                offs[w, c] = cum
                cum += T[w, c]
            call_tiles[b, c] = cum - offs[b * wpair, c]
    tiles = int(cum)

    streams = []
    for m in range(nc_):
        pc = per_core[m]
        dl, cc, lc, vv = pc["dl"], pc["cc"], pc["lc"], pc["vv"]
        win, slot = pc["win"], pc["slot"]
        key = win[dl].astype(np.int64) * 4 + cc
        order = np.argsort(key, kind="stable")
        skey = key[order]
        starts = np.searchsorted(skey, np.arange(nw * 4))
        rank = np.arange(len(skey)) - starts[skey]
        gslot = offs.reshape(-1)[skey] * 128 + rank
        assert (rank < T.reshape(-1)[skey] * 128).all()

        idx_s = np.zeros(tiles * 128, np.int16)
        idx_s[gslot] = lc[order].astype(np.int16)
        gidx = np.tile(np.ascontiguousarray(idx_s.reshape(-1, 16).T), (8, 1))

        # host-built scatter matrices: S_g[k, s] = v for the edge at tile g,
        # partition k, window-slot s (one dense [128, wdest] block per tile)
        scol = np.zeros((128, tiles * wdest), np.float16)
        srow = (gslot % 128).astype(np.int64)
        scolidx = (gslot // 128) * wdest + slot[dl][order].astype(np.int64)
        scol[srow, scolidx] = vv[order].astype(np.float16)

        destmap = -np.ones(nw * wdest, np.int64)
        destmap[win.astype(np.int64) * wdest + slot] = np.arange(npc)
        streams.append(dict(gidx=gidx, scol=scol, destmap=destmap))

    return T, offs, call_tiles, tiles, streams


# ------------------------------------------------------------ device build
def _build_program(T, call_tiles, tiles, cfg):
    nc_ = cfg["n_cores"]
    nw = cfg["nw"]
    wdest = cfg["wdest"]
    wpair = cfg["wpair"]
    nf = cfg["feat"]
    chr_ = cfg["ch_rows"]
    n_batch = nw // wpair

    nc = bacc.Bacc(
        "TRN2",
        debug=False,
        target_bir_lowering=False,
        num_devices=nc_,
        num_swdge_queues=4,
    )
    x16 = nc.dram_tensor("x16", [cfg["n_nodes"], nf], F16, kind="ExternalInput")
    w16 = nc.dram_tensor("w16", [nf, nf], F16, kind="ExternalInput")
    gidx = nc.dram_tensor("gidx", [128, tiles * 8], I16, kind="ExternalInput")
    scol = nc.dram_tensor(
        "scol", [128, tiles * wdest], F16, kind="ExternalInput"
    )
    outT = nc.dram_tensor("outT", [128, nw * wdest], F16, kind="ExternalOutput")

    with tile.TileContext(nc) as tc, ExitStack() as ctx:
        const = ctx.enter_context(tc.tile_pool(name="const", bufs=1))
        gpools = [
            ctx.enter_context(tc.tile_pool(name=f"g{c}", bufs=3))
            for c in range(4)
        ]
        spools = [
            ctx.enter_context(tc.tile_pool(name=f"s{c}", bufs=2))
            for c in range(4)
        ]
        pspool = ctx.enter_context(tc.tile_pool(name="ps", bufs=6, space="PSUM"))
        pzpool = ctx.enter_context(tc.tile_pool(name="pz", bufs=2, space="PSUM"))
        zbpool = ctx.enter_context(tc.tile_pool(name="zb", bufs=6))
        zopool = ctx.enter_context(tc.tile_pool(name="zo", bufs=6))
        idx_t = const.tile([128, tiles * 8], I16)
        # split the idx upload so the first batch's gathers aren't gated on
        # the full 3.4MB transfer
        b0 = int(np.sum(call_tiles[0, :])) * 8
        nc.sync.dma_start(idx_t[:, :b0], gidx[:, :b0])
        w_t = const.tile([128, nf], F16)
        nc.sync.dma_start(w_t[:], w16[:, :])

        for b in range(n_batch):
            if b == 1:
                nc.sync.dma_start(idx_t[:, b0:], gidx[:, b0:])
            ws = list(range(b * wpair, (b + 1) * wpair))
            # per chunk: SWDGE gather of x rows + HWDGE stream of the
            # host-built scatter matrices for the same tile range
            gts = []
            for c in range(4):
                sz = int(call_tiles[b, c])
                gt = gpools[c].tile([128, sz, nf], F16, tag=f"g{c}")
                t0 = int(np.sum(call_tiles[:b]) + np.sum(call_tiles[b, :c]))
                nc.gpsimd.dma_gather(
                    gt[:, :, :],
                    x16[c * chr_ : (c + 1) * chr_, :],
                    idx_t[:, t0 * 8 : (t0 + sz) * 8],
                    sz * 128,
                    sz * 128,
                    nf,
                    queue_num=c,
                    single_packet=False,
                )
                # S stream in two half-call pieces (SBUF footprint)
                h0 = (sz + 1) // 2
                st0 = spools[c].tile([128, h0 * wdest], F16, tag=f"s{c}")
                nc.sync.dma_start(
                    st0[:], scol[:, t0 * wdest : (t0 + h0) * wdest]
                )
                st1 = spools[c].tile([128, (sz - h0) * wdest], F16, tag=f"s{c}")
                nc.sync.dma_start(
                    st1[:], scol[:, (t0 + h0) * wdest : (t0 + sz) * wdest]
                )
                gts.append((gt, (st0, st1, h0), t0))

            for wp in range(wpair):
                w = ws[wp]
                ps = pspool.tile([128, wdest], F32, tag="ps")
                first = True
                last_ct = None
                for c in range(3, -1, -1):
                    if T[w, c] > 0:
                        last_ct = (c, int(T[w, c]) - 1)
                        break
                for c in range(4):
                    gt, (st0, st1, h0), t0 = gts[c]
                    base = int(np.sum([T[ws[i], c] for i in range(wp)]))
                    for t in range(int(T[w, c])):
                        k = base + t
                        if k < h0:
                            s_ap = st0[:, k * wdest : (k + 1) * wdest]
                        else:
                            s_ap = st1[
                                :, (k - h0) * wdest : (k - h0 + 1) * wdest
                            ]
                        nc.tensor.matmul(
                            ps[:],
                            gt[:, k, :],
                            s_ap,
                            start=first,
                            stop=(c, t) == last_ct,
                        )
                        first = False

                zb = zbpool.tile([128, wdest], F16, tag="zb")
                nc.scalar.copy(zb[:], ps[:])
                pz = pzpool.tile([128, wdest], F32, tag="pz")
                nc.tensor.matmul(pz[:], w_t[:], zb[:], start=True, stop=True)
                zo = zopool.tile([128, wdest], F16, tag="zo")
                nc.scalar.copy(zo[:], pz[:])
                nc.sync.dma_start(
                    outT[:, w * wdest : (w + 1) * wdest], zo[:]
                )

    nc.compile()
    return nc


# ------------------------------------------------------------------- entry
def _run(row, col, values, x, weight, cfg, trace=False):
    row = np.asarray(row, dtype=np.int64)
    col = np.asarray(col, dtype=np.int64)
    values = np.asarray(values, dtype=np.float32)
    x = np.asarray(x, dtype=np.float32)
    weight = np.asarray(weight, dtype=np.float32)

    nc_ = cfg["n_cores"]
    npc = cfg["npc"]

    T, offs, call_tiles, tiles, streams = _preprocess(row, col, values, cfg)
    nc = _build_program(T, call_tiles, tiles, cfg)

    x16 = x.astype(np.float16)
    w16 = weight.astype(np.float16)

    in_maps = []
    for m in range(nc_):
        st = streams[m]
        in_maps.append(
            dict(x16=x16, w16=w16, gidx=st["gidx"], scol=st["scol"])
        )

    res = bass_utils.run_bass_kernel_spmd(
        nc, in_maps, core_ids=list(range(nc_)), trace=trace
    )

    out = np.zeros((cfg["n_nodes"], cfg["feat"]), np.float32)
    for m in range(nc_):
        oT = res.results[m]["outT"].astype(np.float32)  # [128, nw*wdest]
        dm = streams[m]["destmap"]
        valid = dm >= 0
        out[m * npc + dm[valid]] = oT[:, valid].T
    return out, res


def kernel(row, col, values, x, weight):
    out, _ = _run(row, col, values, x, weight, DEFAULT_CFG)
    return out

